# revision 9
# baseline (speedup 1.0000x reference)
"""BiLSTM-CRF (word+char CNN, 2-layer BiLSTM, Viterbi decode) on 8 Trainium2 cores.

Data-parallel: each core handles 16 of the 128 sequences end-to-end.
All math fp32. Feature-major x^T built via indirect-DMA gathers + PE transposes.
"""
import sys

sys.path.insert(0, "/opt/trn_rl_repo")

import numpy as np

import concourse.bass as bass
import concourse.mybir as mybir
import concourse.tile as tile
from concourse import bacc
from concourse.bass import AP, IndirectOffsetOnAxis
from concourse.bass_utils import run_bass_kernel_spmd
from concourse.masks import make_identity

FP = mybir.dt.float32
I32 = mybir.dt.int32
AF = mybir.ActivationFunctionType
OP = mybir.AluOpType

# model dims
V, E, H, L, CV = 50000, 300, 256, 9, 100
B, T, M = 128, 128, 16
CE, NF = 30, 50
NCORES = 8
BS = B // NCORES          # 16 sequences per core
W = BS * T                # 2048 words per core
CHARS = W * M             # 32768 chars per core
G4 = 4 * H                # 1024
D0 = E + 2 * NF           # 400
D1 = 2 * H                # 512

OFF_TRANS, OFF_IOTA81, OFF_IOTA9, OFF_START, OFF_END = 0, 81, 162, 171, 180


def _bcast(ap: AP, npart: int) -> AP:
    """Broadcast a DRAM row vector across npart partitions."""
    return AP(tensor=ap.tensor, offset=ap.offset, ap=[[0, npart]] + list(ap.ap))


def _bc_mid(ap: AP, n: int) -> AP:
    """[p, k] -> [p, n, k] with 0-stride middle dim."""
    a = list(ap.ap)
    return AP(tensor=ap.tensor, offset=ap.offset, ap=[a[0], [0, n]] + a[1:])


def _bc_last(ap: AP, n: int) -> AP:
    """[p, k] -> [p, k, n] with 0-stride last dim."""
    a = list(ap.ap)
    return AP(tensor=ap.tensor, offset=ap.offset, ap=a + [[0, n]])


def _bc_scalar(ap: AP, n: int) -> AP:
    """[p, 1] -> [p, n] with 0-stride free dim."""
    return AP(tensor=ap.tensor, offset=ap.offset, ap=[ap.ap[0], [0, n]])


def build_nc():
    nc = bacc.Bacc("TRN2", target_bir_lowering=False)

    word_ids = nc.dram_tensor("word_ids", [W], I32, kind="ExternalInput")
    char_ids = nc.dram_tensor("char_ids", [CHARS], I32, kind="ExternalInput")
    word_emb = nc.dram_tensor("word_emb", [V, E], FP, kind="ExternalInput")
    char_emb = nc.dram_tensor("char_emb", [CV, CE], FP, kind="ExternalInput")
    w23 = nc.dram_tensor("w23", [96, 100], FP, kind="ExternalInput")
    w23_t14 = nc.dram_tensor("w23_t14", [62, 50], FP, kind="ExternalInput")
    b23 = nc.dram_tensor("b23", [100], FP, kind="ExternalInput")
    wih, whh, bias = {}, {}, {}
    for l, din in ((0, D0), (1, D1)):
        for d in "fb":
            wih[l, d] = nc.dram_tensor(f"wih{l}{d}", [din, G4], FP, kind="ExternalInput")
            whh[l, d] = nc.dram_tensor(f"whh{l}{d}", [H, G4], FP, kind="ExternalInput")
            bias[l, d] = nc.dram_tensor(f"bias{l}{d}", [G4], FP, kind="ExternalInput")
    fc_wT = nc.dram_tensor("fc_wT", [D1, L], FP, kind="ExternalInput")
    fc_b = nc.dram_tensor("fc_b", [L], FP, kind="ExternalInput")
    vit = nc.dram_tensor("vit", [189], FP, kind="ExternalInput")
    paths_out = nc.dram_tensor("paths", [BS, T], I32, kind="ExternalOutput")

    with tile.TileContext(nc) as tc:
        _emit(nc, tc, word_ids, char_ids, word_emb, char_emb, w23, w23_t14, b23,
              wih, whh, bias, fc_wT, fc_b, vit, paths_out)
    nc.finalize()
    return nc


def _emit(nc, tc, word_ids, char_ids, word_emb, char_emb, w23, w23_t14, b23,
          wih, whh, bias, fc_wT, fc_b, vit, paths_out):
    import contextlib
    ctx = contextlib.ExitStack()
    singles = ctx.enter_context(tc.tile_pool(name="singles", bufs=1))
    dram = ctx.enter_context(tc.tile_pool(name="dram", bufs=1, space="DRAM"))

    ident = singles.tile([128, 128], FP, tag="ident")
    make_identity(nc, ident)

    # persistent feature-major buffers
    xt0 = singles.tile([128, W], FP, tag="xt0")
    xt1 = singles.tile([128, W], FP, tag="xt1")
    xt2 = singles.tile([44, W], FP, tag="xt2")
    xt3 = singles.tile([100, W], FP, tag="xt3")
    h0T = {d: singles.tile([128, 2, W], FP, tag=f"h0T{d}", name=f"h0T{d}") for d in "fb"}
    h1T = {d: singles.tile([128, 2, W], FP, tag=f"h1T{d}", name=f"h1T{d}") for d in "fb"}
    xs_dram = {(l, d): dram.tile([T, BS, G4], FP, tag=f"xs{l}{d}", name=f"xs{l}{d}")
               for l in (0, 1) for d in "fb"}

    # ==================== stage P: gathers, transposes, char CNN ====================
    with tc.tile_pool(name="prep", bufs=4) as prep, \
         tc.tile_pool(name="psum_p", bufs=2, space="PSUM") as psum_p, \
         tc.tile_pool(name="ct3p", bufs=2) as ct3p:

        # word embeddings -> xt0/xt1/xt2
        for c in range(W // 128):
            idx = prep.tile([128, 1], I32, tag="widx")
            nc.sync.dma_start(out=idx, in_=word_ids[bass.ts(c, 128)].rearrange("(p o) -> p o", o=1))
            g = prep.tile([128, E], FP, tag="wrows")
            nc.gpsimd.indirect_dma_start(
                out=g[:], out_offset=None, in_=word_emb[:],
                in_offset=IndirectOffsetOnAxis(ap=idx[:, :1], axis=0))
            for s, (dst, nr) in enumerate(((xt0, 128), (xt1, 128), (xt2, 44))):
                pt = psum_p.tile([128, 128], FP, tag="tp")
                nc.tensor.transpose(out=pt[:nr, :], in_=g[:, s * 128: s * 128 + nr], identity=ident[:])
                if s % 2 == 0:
                    nc.scalar.copy(out=dst[:, bass.ts(c, 128)], in_=pt[:nr, :])
                else:
                    nc.vector.tensor_copy(out=dst[:, bass.ts(c, 128)], in_=pt[:nr, :])

        # char CNN consts
        w23_sb = singles.tile([96, 100], FP, tag="w23")
        nc.sync.dma_start(out=w23_sb, in_=w23[:, :])
        w23t14_sb = singles.tile([62, 50], FP, tag="w23t14")
        nc.sync.dma_start(out=w23t14_sb, in_=w23_t14[:, :])
        b23_sb = singles.tile([100, 1], FP, tag="b23")
        nc.sync.dma_start(out=b23_sb, in_=b23[:].rearrange("(p o) -> p o", o=1))

        QW, QC = 512, 512 * M  # words/chars per quarter
        for q in range(CHARS // QC):
            ct3 = ct3p.tile([96, QC], FP, tag="ct3")  # rows 0-29 CT, 32-61 sh1, 64-93 sh2
            for gp in range(3):
                nc.vector.memset(ct3[32 * gp: 32 * gp + 32, :], 0.0)
            for gi in range(QC // 512):
                base = q * QC + gi * 512
                gd = prep.tile([128, 4, 32], FP, tag="crows")
                nc.vector.memset(gd[:, :, CE:32], 0.0)
                for b4 in range(4):
                    idx = prep.tile([128, 1], I32, tag="cidx")
                    nc.sync.dma_start(
                        out=idx,
                        in_=char_ids[base + b4 * 128: base + (b4 + 1) * 128].rearrange("(p o) -> p o", o=1))
                    nc.gpsimd.indirect_dma_start(
                        out=gd[:, b4, :CE], out_offset=None, in_=char_emb[:],
                        in_offset=IndirectOffsetOnAxis(ap=idx[:, :1], axis=0))
                pt = psum_p.tile([128, 128], FP, tag="tp")
                nc.tensor.transpose(out=pt[:], in_=gd[:].rearrange("p a b -> p (a b)"), identity=ident[:])
                stg = prep.tile([128, 128], FP, tag="cstg")
                nc.vector.tensor_copy(out=stg[:], in_=pt[:])
                for b4 in range(4):
                    cbase = gi * 512 + b4 * 128
                    for s in range(3):
                        lo = cbase - s
                        nlo = max(lo, 0)
                        cnt = 128 - (nlo - lo)
                        nc.sync.dma_start(
                            out=ct3[32 * s: 32 * s + CE, nlo: nlo + cnt],
                            in_=stg[32 * b4: 32 * b4 + CE, nlo - lo: nlo - lo + cnt])
            for wc in range(QW // 32):
                colbase = wc * 32 * M
                ps1 = psum_p.tile([100, 32, 14], FP, tag="cps1")
                rhs = ct3[:, colbase: colbase + 32 * M].rearrange("p (n t) -> p n t", t=M)[:, :, 0:14]
                nc.tensor.matmul(out=ps1[:], lhsT=w23_sb[:, :], rhs=rhs, start=True, stop=True)
                ps2 = psum_p.tile([50, 32], FP, tag="cps2")
                rhs2 = ct3[0:62, colbase: colbase + 32 * M].rearrange("p (n t) -> p n t", t=M)[:, :, 14]
                nc.tensor.matmul(out=ps2[:], lhsT=w23t14_sb[:, :], rhs=rhs2, start=True, stop=True)
                red = prep.tile([100, 32], FP, tag="cred")
                nc.vector.tensor_reduce(out=red[:], in_=ps1[:], axis=mybir.AxisListType.X, op=OP.max)
                nc.vector.tensor_tensor(out=red[0:50, :], in0=red[0:50, :], in1=ps2[:], op=OP.max)
                nc.scalar.activation(out=xt3[:, q * QW + wc * 32: q * QW + (wc + 1) * 32],
                                     in_=red[:], func=AF.Relu, bias=b23_sb[:, 0:1])

    # ==================== projections ====================
    def emit_proj(projp, psum_j, l, d, slabs):
        wt = projp.tile([128, 4, G4], FP, tag="wt")
        roff = 0
        for k, (st, nr, ss) in enumerate(slabs):
            nc.sync.dma_start(out=wt[0:nr, k, :], in_=wih[l, d][roff:roff + nr, :])
            roff += nr
        bb = projp.tile([128, G4], FP, tag="bb")
        nc.sync.dma_start(out=bb, in_=_bcast(bias[l, d][:], 128))
        for b in range(BS):
            for nh in range(2):
                ps = psum_j.tile([128, 512], FP, tag="pps")
                for k, (st, nr, ss) in enumerate(slabs):
                    sl = st[:, :] if ss is None else st[:, ss, :]
                    nc.tensor.matmul(out=ps[:], lhsT=sl[0:nr, bass.ts(b, T)],
                                     rhs=wt[0:nr, k, bass.ts(nh, 512)],
                                     start=(k == 0), stop=(k == len(slabs) - 1))
                stg = projp.tile([128, 512], FP, tag="pstg")
                nc.vector.tensor_tensor(out=stg[:], in0=ps[:], in1=bb[:, bass.ts(nh, 512)], op=OP.add)
                nc.sync.dma_start(out=xs_dram[l, d][:, b, bass.ts(nh, 512)], in_=stg[:])

    # ==================== LSTM ====================
    def emit_lstm(recp, xsp, psum_r, l, d, houtT):
        whh_sb = recp.tile([128, 2, G4], FP, tag=f"whh{d}")
        for k in range(2):
            nc.sync.dma_start(out=whh_sb[:, k, :], in_=whh[l, d][bass.ts(k, 128), :])
        hT = recp.tile([128, 2, BS], FP, tag=f"hT{d}")
        c_st = recp.tile([BS, H], FP, tag=f"c{d}")
        nc.vector.memset(hT[:], 0.0)
        nc.vector.memset(c_st[:], 0.0)
        gates = recp.tile([BS, G4], FP, tag=f"g{d}")
        tmp = recp.tile([BS, 2, H], FP, tag=f"tmp{d}")
        h_sb = recp.tile([BS, H], FP, tag=f"h{d}")
        ho_tb = houtT[:].rearrange("p s (b t) -> p s t b", t=T)
        for step in range(T):
            t = step if d == "f" else T - 1 - step
            xs_t = xsp.tile([BS, G4], FP, tag=f"xs{d}")
            nc.sync.dma_start(out=xs_t, in_=xs_dram[l, d][t, :, :])
            ps = psum_r.tile([BS, G4], FP, tag=f"ps{d}")
            for nh in range(2):
                for k in range(2):
                    nc.tensor.matmul(out=ps[:, bass.ts(nh, 512)], lhsT=hT[:, k, :],
                                     rhs=whh_sb[:, k, bass.ts(nh, 512)],
                                     start=(k == 0), stop=(k == 1))
            nc.vector.tensor_tensor(out=gates[:, 0:512], in0=ps[:, 0:512], in1=xs_t[:, 0:512], op=OP.add)
            nc.scalar.activation(out=gates[:, 0:512], in_=gates[:, 0:512], func=AF.Sigmoid)
            nc.vector.tensor_tensor(out=gates[:, 512:1024], in0=ps[:, 512:1024],
                                    in1=xs_t[:, 512:1024], op=OP.add)
            nc.scalar.activation(out=gates[:, 512:768], in_=gates[:, 512:768], func=AF.Tanh)
            nc.scalar.activation(out=gates[:, 768:1024], in_=gates[:, 768:1024], func=AF.Sigmoid)
            nc.vector.tensor_tensor(out=tmp[:, 0, :], in0=gates[:, 0:256], in1=gates[:, 512:768], op=OP.mult)
            nc.vector.tensor_tensor(out=c_st[:], in0=gates[:, 256:512], in1=c_st[:], op=OP.mult)
            nc.vector.tensor_tensor(out=c_st[:], in0=c_st[:], in1=tmp[:, 0, :], op=OP.add)
            nc.scalar.activation(out=tmp[:, 1, :], in_=c_st[:], func=AF.Tanh)
            nc.vector.tensor_tensor(out=h_sb[:], in0=gates[:, 768:1024], in1=tmp[:, 1, :], op=OP.mult)
            pt = psum_r.tile([128, 2, BS], FP, tag=f"pt{d}")
            for s in range(2):
                nc.tensor.transpose(out=pt[:, s, :], in_=h_sb[:, bass.ts(s, 128)], identity=ident[0:BS, 0:BS])
            nc.vector.tensor_copy(out=hT[:], in_=pt[:])
            nc.scalar.copy(out=ho_tb[:, :, t, :], in_=pt[:])

    x_slabs = [(xt0, 128, None), (xt1, 128, None), (xt2, 44, None), (xt3, 100, None)]
    h0_slabs = [(h0T["f"], 128, 0), (h0T["f"], 128, 1), (h0T["b"], 128, 0), (h0T["b"], 128, 1)]

    with tc.tile_pool(name="projp", bufs=2) as projp, \
         tc.tile_pool(name="psum_j", bufs=2, space="PSUM") as psum_j:
        for d in "fb":
            emit_proj(projp, psum_j, 0, d, x_slabs)

    with tc.tile_pool(name="recp", bufs=1) as recp, \
         tc.tile_pool(name="xsp", bufs=6) as xsp, \
         tc.tile_pool(name="psum_r", bufs=1, space="PSUM") as psum_r:
        for d in "fb":
            emit_lstm(recp, xsp, psum_r, 0, d, h0T[d])

    with tc.tile_pool(name="projp1", bufs=2) as projp, \
         tc.tile_pool(name="psum_j1", bufs=2, space="PSUM") as psum_j:
        for d in "fb":
            emit_proj(projp, psum_j, 1, d, h0_slabs)

    with tc.tile_pool(name="recp1", bufs=1) as recp, \
         tc.tile_pool(name="xsp1", bufs=6) as xsp, \
         tc.tile_pool(name="psum_r1", bufs=1, space="PSUM") as psum_r:
        for d in "fb":
            emit_lstm(recp, xsp, psum_r, 1, d, h1T[d])

    # ==================== emissions ====================
    em_b = singles.tile([BS, T, L], FP, tag="em_b")
    with tc.tile_pool(name="emp", bufs=2) as emp, \
         tc.tile_pool(name="psum_e", bufs=2, space="PSUM") as psum_e:
        fcw_sb = emp.tile([128, 4, L], FP, tag="fcw")
        for k in range(4):
            nc.sync.dma_start(out=fcw_sb[:, k, :], in_=fc_wT[bass.ts(k, 128), :])
        fcb_sb = emp.tile([L, 1], FP, tag="fcb")
        nc.sync.dma_start(out=fcb_sb, in_=fc_b[:].rearrange("(p o) -> p o", o=1))
        h1_slabs = [(h1T["f"], 0), (h1T["f"], 1), (h1T["b"], 0), (h1T["b"], 1)]
        emT = emp.tile([L, W], FP, tag="emT")
        for ncol in range(W // 512):
            ps = psum_e.tile([L, 512], FP, tag="emps")
            for k, (ht, ss) in enumerate(h1_slabs):
                nc.tensor.matmul(out=ps[:], lhsT=fcw_sb[:, k, :], rhs=ht[:, ss, bass.ts(ncol, 512)],
                                 start=(k == 0), stop=(k == 3))
            nc.scalar.activation(out=emT[:, bass.ts(ncol, 512)], in_=ps[:],
                                 func=AF.Identity, bias=fcb_sb[:, 0:1])
        for lbl in range(L):
            src = emT[lbl: lbl + 1, :].rearrange("o (b t) -> o b t", b=BS)
            dst = em_b[:].rearrange("p t l -> p l t")[:, lbl, :]
            nc.sync.dma_start(out=dst, in_=src)

    # ==================== viterbi ====================
    with tc.tile_pool(name="vitp", bufs=1) as vitp:
        transT = vitp.tile([BS, 9, L], FP, tag="transT")
        nc.sync.dma_start(out=transT, in_=_bcast(vit[OFF_TRANS:OFF_TRANS + 81], BS))
        iota81 = vitp.tile([BS, 81], FP, tag="iota81")
        nc.sync.dma_start(out=iota81, in_=_bcast(vit[OFF_IOTA81:OFF_IOTA81 + 81], BS))
        iota9 = vitp.tile([BS, L], FP, tag="iota9")
        nc.sync.dma_start(out=iota9, in_=_bcast(vit[OFF_IOTA9:OFF_IOTA9 + 9], BS))
        startv = vitp.tile([BS, L], FP, tag="startv")
        nc.sync.dma_start(out=startv, in_=_bcast(vit[OFF_START:OFF_START + 9], BS))
        endv = vitp.tile([BS, L], FP, tag="endv")
        nc.sync.dma_start(out=endv, in_=_bcast(vit[OFF_END:OFF_END + 9], BS))

        v = vitp.tile([BS, L], FP, tag="v")
        s_t = vitp.tile([BS, 9, L], FP, tag="s_t")
        mx = vitp.tile([BS, L], FP, tag="mx")
        eq = vitp.tile([BS, 9, L], FP, tag="eq")
        bps = vitp.tile([BS, T - 1, L], FP, tag="bps")
        paths_f = vitp.tile([BS, T], FP, tag="paths_f")
        paths_i = vitp.tile([BS, T], I32, tag="paths_i")

        nc.vector.tensor_tensor(out=v[:], in0=startv[:], in1=em_b[:, 0, :], op=OP.add)
        for t in range(1, T):
            nc.vector.tensor_tensor(out=s_t[:], in0=_bc_mid(v[:], 9), in1=transT[:], op=OP.add)
            nc.vector.tensor_reduce(out=mx[:], in_=s_t[:], axis=mybir.AxisListType.X, op=OP.max)
            nc.vector.tensor_tensor(out=eq[:], in0=s_t[:], in1=_bc_last(mx[:], L), op=OP.is_ge)
            nc.vector.tensor_tensor(out=eq[:].rearrange("p a b -> p (a b)"), in0=eq[:].rearrange("p a b -> p (a b)"),
                                    in1=iota81[:], op=OP.mult)
            nc.vector.tensor_reduce(out=bps[:, t - 1, :], in_=eq[:], axis=mybir.AxisListType.X, op=OP.min)
            nc.vector.tensor_tensor(out=v[:], in0=mx[:], in1=em_b[:, t, :], op=OP.add)
        nc.vector.tensor_tensor(out=v[:], in0=v[:], in1=endv[:], op=OP.add)
        nc.vector.tensor_reduce(out=mx[:, 0:1], in_=v[:], axis=mybir.AxisListType.X, op=OP.max)
        nc.vector.tensor_tensor(out=eq[:, 0, :], in0=v[:],
                                in1=_bc_scalar(mx[:, 0:1], L), op=OP.is_ge)
        nc.vector.tensor_tensor(out=eq[:, 0, :], in0=eq[:, 0, :], in1=iota9[:], op=OP.mult)
        nc.vector.tensor_reduce(out=paths_f[:, T - 1: T], in_=eq[:, 0, :],
                                axis=mybir.AxisListType.X, op=OP.min)
        tag_ap = paths_f[:, T - 1: T]
        for t in range(T - 1, 0, -1):
            nc.vector.scalar_tensor_tensor(out=eq[:, 0, :], in0=iota9[:], scalar=tag_ap,
                                           in1=bps[:, t - 1, :], op0=OP.is_equal, op1=OP.mult)
            nc.vector.tensor_reduce(out=paths_f[:, t - 1: t], in_=eq[:, 0, :],
                                    axis=mybir.AxisListType.X, op=OP.add)
            tag_ap = paths_f[:, t - 1: t]
        nc.vector.tensor_scalar_add(paths_f[:], paths_f[:], 9.0)
        nc.vector.tensor_copy(out=paths_i[:], in_=paths_f[:])
        nc.sync.dma_start(out=paths_out[:, :], in_=paths_i[:])

    ctx.close()


# ============================ host side ============================
_NC_CACHE = None


def _get_nc():
    global _NC_CACHE
    if _NC_CACHE is None:
        _NC_CACHE = build_nc()
    return _NC_CACHE


def _prep_consts(inputs):
    f32 = np.float32
    out = {}
    out["word_emb"] = np.ascontiguousarray(np.asarray(inputs["word_emb"], f32))
    out["char_emb"] = np.ascontiguousarray(np.asarray(inputs["char_emb"], f32))
    w23 = np.zeros((96, 100), f32)
    w23t14 = np.zeros((62, 50), f32)
    c2 = np.asarray(inputs["conv2_w"], f32)
    c3 = np.asarray(inputs["conv3_w"], f32)
    for s in range(2):
        w23[32 * s:32 * s + CE, 0:50] = c2[:, :, s].T
        w23t14[32 * s:32 * s + CE, :] = c2[:, :, s].T
    for s in range(3):
        w23[32 * s:32 * s + CE, 50:100] = c3[:, :, s].T
    out["w23"] = w23
    out["w23_t14"] = w23t14
    out["b23"] = np.concatenate([np.asarray(inputs["conv2_b"], f32), np.asarray(inputs["conv3_b"], f32)])
    for l in (0, 1):
        for d in "fb":
            out[f"wih{l}{d}"] = np.ascontiguousarray(np.asarray(inputs[f"w_ih_l{l}{d}"], f32).T)
            out[f"whh{l}{d}"] = np.ascontiguousarray(np.asarray(inputs[f"w_hh_l{l}{d}"], f32).T)
            out[f"bias{l}{d}"] = (np.asarray(inputs[f"b_ih_l{l}{d}"], f32)
                                  + np.asarray(inputs[f"b_hh_l{l}{d}"], f32))
    out["fc_wT"] = np.ascontiguousarray(np.asarray(inputs["fc_w"], f32).T)
    out["fc_b"] = np.asarray(inputs["fc_b"], f32)
    trans = np.asarray(inputs["transitions"], f32)
    ii = np.arange(9, dtype=f32)
    out["vit"] = np.concatenate([
        np.ascontiguousarray(trans.T).reshape(-1),  # transT[j*9+i] = trans[i, j]
        np.tile(ii - 9.0, 9),
        ii - 9.0,
        np.asarray(inputs["start_transitions"], f32),
        np.asarray(inputs["end_transitions"], f32),
    ])
    return out


def kernel(**inputs):
    nc = _get_nc()
    consts = _prep_consts(inputs)
    wid = np.asarray(inputs["word_ids"]).astype(np.int32).reshape(B, T)
    cid = np.asarray(inputs["char_ids"]).astype(np.int32).reshape(B, T, M)
    in_maps = []
    for c in range(NCORES):
        m = dict(consts)
        m["word_ids"] = np.ascontiguousarray(wid[c * BS:(c + 1) * BS].reshape(-1))
        m["char_ids"] = np.ascontiguousarray(cid[c * BS:(c + 1) * BS].reshape(-1))
        in_maps.append(m)
    res = run_bass_kernel_spmd(nc, in_maps, list(range(NCORES)))
    paths = np.concatenate([res.results[c]["paths"] for c in range(NCORES)], axis=0)
    return paths.astype(np.int32)


if __name__ == "__main__":
    _get_nc()
    print("built ok")
